# revision 2
# baseline (speedup 1.0000x reference)
"""Trainium2 Bass kernel for nn_BlockFourierCirculant — v3.

Same three-matmul-stage dataflow as the baseline (batch-major spectrum with
xbar DMA transposes between stages), with the descriptor-economics fixed:

  * x / y travel HBM<->SBUF as per-chunk packed tensors with 16 KiB
    contiguous per-partition rows -> 128 descriptors per 2 MiB transfer
    (the baseline's strided layout generated 4096 512-B descriptors per
    transfer, costing ~213 us of Q7 SWDGE descriptor-generation time).
  * The xbar transposes (~10.7 us of HWDGE issue time each) are split
    across both HWDGE issuing engines (SP gets 7, ACT gets 1).
  * PSUM evacuation copies are rebalanced DVE-heavy (4:3) since ACT also
    pays transpose-issue time.

Stages per chunk of 256 batch rows:
  s1  (x stationary):  Xhat[b, slot]  = x[s, b].T @ T1[s, slot]
  xbar transpose:      (b, (g,i,u)) -> ((i,u), b)  per 4-bin group g
  s2  (X stationary):  Yhat[b, (o,u')] = Xg[(i,u), b].T @ W2[(i,u), (o,u')]
  xbar transpose:      (b, (o,kch,p4)) -> (p4, b)
  s3  (B3 stationary): y[t, b]        = B3[p4, t].T @ Y[p4, b]
"""

import numpy as np

import concourse.bacc as bacc
import concourse.bass as bass
import concourse.mybir as mybir
import concourse.tile as tile
from concourse.bass_utils import run_bass_kernel_spmd

# ---------------------------------------------------------------- constants
BATCH = 8192
FEAT = 4096
BLOCK = 256
NBLK = 16
NSLOT = 256
NGRP = 32
N_CORES = 8
BC = BATCH // N_CORES  # 1024
NB = 256
NCHUNK = BC // NB  # 4

F32 = mybir.dt.float32
F16 = mybir.dt.float16


# ------------------------------------------------------------- host matrices
def _slot_map():
    m = [(0, 0), (0, 128)]
    for k in range(1, 128):
        m.append((0, k))
        m.append((1, k))
    return m


_SLOTS = _slot_map()


def build_t1():
    s = np.arange(BLOCK)
    T1 = np.zeros((BLOCK, NSLOT), dtype=np.float64)
    for j, (comp, k) in enumerate(_SLOTS):
        ang = 2.0 * np.pi * k * s / BLOCK
        T1[:, j] = np.cos(ang) if comp == 0 else -np.sin(ang)
    return T1


def build_b3():
    t = np.arange(BLOCK)
    B3 = np.zeros((NSLOT, BLOCK), dtype=np.float64)
    for j, (comp, k) in enumerate(_SLOTS):
        w = 1.0 if k in (0, 128) else 2.0
        ang = 2.0 * np.pi * k * t / BLOCK
        B3[j] = (w * np.cos(ang) if comp == 0 else -w * np.sin(ang)) / BLOCK
    return B3


def build_w2(W_real, W_imag):
    Wr = np.asarray(W_real, dtype=np.float64)
    Wi = np.asarray(W_imag, dtype=np.float64)
    W2 = np.zeros((NGRP, 128, 128), dtype=np.float64)
    for g in range(NGRP):
        for u in range(8):
            comp_u, k_u = _SLOTS[8 * g + u]
            for up in range(8):
                comp_up, k_up = _SLOTS[8 * g + up]
                if k_u != k_up:
                    continue
                k = k_u
                if comp_u == 0 and comp_up == 0:
                    coef = Wr[:, :, k]
                elif comp_u == 1 and comp_up == 0:
                    coef = -Wi[:, :, k]
                elif comp_u == 0 and comp_up == 1:
                    coef = Wi[:, :, k]
                else:
                    coef = Wr[:, :, k]
                W2[g, u::8, up::8] = coef.T  # [i, o]
    return W2


def pack_t1(T1):
    return np.ascontiguousarray(T1.reshape(2, 128, NSLOT).transpose(1, 0, 2)).astype(
        np.float16
    )


def pack_b3(B3):
    return np.ascontiguousarray(B3.reshape(2, 128, BLOCK).transpose(1, 0, 2)).astype(
        np.float16
    )


def pack_w2(W2):
    return np.ascontiguousarray(W2.transpose(1, 0, 2)).astype(np.float16)


# ------------------------------------------------------------- device kernel
def emit_kernel(tc, outs, ins, n_chunks=NCHUNK):
    nc = tc.nc

    with (
        tc.tile_pool(name="wpool", bufs=1) as wpool,
        tc.tile_pool(name="xpool", bufs=2) as xpool,
        tc.tile_pool(name="spec", bufs=3) as spec,
        tc.tile_pool(name="yst", bufs=2) as ystp,
        tc.tile_pool(name="psum", bufs=8, space="PSUM") as pspool,
    ):
        t1_sb = wpool.tile([128, 2, NSLOT], F16)
        w2_sb = wpool.tile([128, NGRP, 128], F16)
        b3_sb = wpool.tile([128, 2, BLOCK], F16)
        # weights ride the otherwise-idle ACT HWDGE ring so they don't
        # serialize with the x(0) load on the gpsimd SWDGE queue
        nc.scalar.dma_start(out=t1_sb, in_=ins["t1"])
        nc.scalar.dma_start(out=w2_sb, in_=ins["w2"])
        nc.scalar.dma_start(out=b3_sb, in_=ins["b3"])

        ncopy = 0  # alternate evacuation copies between DVE and ACT

        def evac(dst, src):
            nonlocal ncopy
            if ncopy % 2 == 0:
                nc.vector.tensor_copy(out=dst, in_=src)
            else:
                nc.scalar.copy(out=dst, in_=src)
            ncopy += 1

        ntrans = 0  # alternate transpose issue between the two HWDGE engines

        def transpose(dst, src):
            nonlocal ntrans
            eng = nc.sync  # single ring while testing xbar concurrency safety
            eng.dma_start_transpose(out=dst, in_=src)
            ntrans += 1

        xg_t = {}
        yom_t = {}

        def stage1(n):
            """x load + per-block forward DFT + shuffle 1."""
            x_sb = xpool.tile([128, 32, NB], F16, tag="x", name=f"x_{n}")
            nc.gpsimd.dma_start(out=x_sb, in_=ins["xTp"][n])

            # xbt[b, bsub, g, i, u] = Xhat[b', i, slot 8g+u]
            # bsub-outer so each transpose half launches after its 8 evacs
            xbt = spec.tile([128, 2, NGRP, NBLK, 8], F16, tag="xbt", name=f"xbt_{n}")
            xg = spec.tile([128, 2, NGRP, 128], F16, tag="xg", name=f"xg_{n}")
            for bsub in range(2):
                for i in range(0, NBLK, 2):
                    ps = pspool.tile([128, 2, NSLOT], F32, tag="ps", name=f"ps1_{n}")
                    for ip in range(2):
                        for kc in range(2):
                            nc.tensor.matmul(
                                ps[:, ip, :],
                                x_sb[
                                    :, 2 * (i + ip) + kc, bsub * 128 : bsub * 128 + 128
                                ],
                                t1_sb[:, kc, :],
                                start=(kc == 0),
                                stop=(kc == 1),
                            )
                    evac(
                        xbt[:, bsub, :, i : i + 2, :],
                        ps.rearrange("p i (g u) -> p g i u", u=8),
                    )
                # shuffle 1 half: xbar transpose to spectrum-major
                transpose(xg[:, bsub], xbt[:, bsub])
            xg_t[n] = xg

        def stage2(n):
            """per-bin spectral mix + shuffle 2."""
            xg = xg_t.pop(n)
            ybt = spec.tile(
                [128, 2, NBLK, 2, 16, 8], F16, tag="xbt", name=f"ybt_{n}"
            )
            yom = spec.tile([128, 2, NBLK, 2, 128], F16, tag="xg", name=f"yom_{n}")
            for bsub in range(2):
                for g in range(0, NGRP, 4):
                    kch, gp = divmod(g, 16)
                    ps = pspool.tile([128, 4, 128], F32, tag="ps", name=f"ps2_{n}")
                    for q in range(4):
                        nc.tensor.matmul(
                            ps[:, q, :],
                            xg[:, bsub, g + q, :],
                            w2_sb[:, g + q, :],
                            start=True,
                            stop=True,
                        )
                    evac(
                        ybt[:, bsub, :, kch, gp : gp + 4, :],
                        ps.rearrange("p q (o u) -> p o q u", u=8),
                    )
                # shuffle 2 half: xbar transpose to slot-major per block
                transpose(yom[:, bsub], ybt[:, bsub])
            yom_t[n] = yom

        def stage3(n):
            """per-block inverse DFT + store."""
            yom = yom_t.pop(n)
            ybig = ystp.tile([128, 32, NB], F16, tag="ybig", name=f"ybig_{n}")
            for ob in range(0, NBLK, 4):
                pss = [
                    pspool.tile([128, 2, NB], F32, tag="ps", name=f"ps3_{n}_{ob}_{j}")
                    for j in range(4)
                ]
                for mch in range(2):
                    for kch in range(2):
                        for j in range(4):
                            nc.tensor.matmul(
                                pss[j][:, mch, :],
                                b3_sb[:, kch, mch * 128 : mch * 128 + 128],
                                yom[:, :, ob + j, kch, :],
                                start=(kch == 0),
                                stop=(kch == 1),
                            )
                for j in range(4):
                    evac(ybig[:, 2 * (ob + j) : 2 * (ob + j) + 2, :], pss[j])
            nc.gpsimd.dma_start(out=outs["yTp"][n], in_=ybig)

        # emission (= priority) order per round, hand-scheduled around the
        # transpose ring (the binding resource at ~4.9us issue per MB, which
        # the tile cost model underestimates): s1(k) first (feeds T1(k)),
        # then s2(k-1) early so T2(k-1) inputs are ready before the ring
        # reaches it, then s3(k-2) (its turn completed last round).
        for k in range(n_chunks + 2):
            if k < n_chunks:
                stage1(k)
            if 0 <= k - 1 < n_chunks:
                stage2(k - 1)
            if 0 <= k - 2 < n_chunks:
                stage3(k - 2)


# ------------------------------------------------------------ host interface
_CACHED = {}


def make_inputs(W_real, W_imag):
    return {
        "t1": pack_t1(build_t1()),
        "w2": pack_w2(build_w2(W_real, W_imag)),
        "b3": pack_b3(build_b3()),
    }


def _build_bass():
    if "nc" in _CACHED:
        return _CACHED["nc"]
    nc = bacc.Bacc("TRN2", target_bir_lowering=False, debug=False)
    ins = {
        "xTp": nc.dram_tensor(
            "xTp", [NCHUNK, 128, 32, NB], F16, kind="ExternalInput"
        ).ap(),
        "t1": nc.dram_tensor("t1", [128, 2, NSLOT], F16, kind="ExternalInput").ap(),
        "w2": nc.dram_tensor("w2", [128, NGRP, 128], F16, kind="ExternalInput").ap(),
        "b3": nc.dram_tensor("b3", [128, 2, BLOCK], F16, kind="ExternalInput").ap(),
    }
    outs = {
        "yTp": nc.dram_tensor(
            "yTp", [NCHUNK, 128, 32, NB], F16, kind="ExternalOutput"
        ).ap()
    }
    with tile.TileContext(nc) as tc:
        emit_kernel(tc, outs, ins, NCHUNK)
    nc.compile()
    _CACHED["nc"] = nc
    return nc


def run_sharded(x, W_real, W_imag, trace=False):
    x = np.asarray(x, dtype=np.float32)
    w = make_inputs(W_real, W_imag)

    in_maps = []
    for c in range(N_CORES):
        xc = x[c * BC:(c + 1) * BC].astype(np.float16)  # [1024, 4096]
        # xTp[n, p, r, b] = xc[n*NB + b, r*128 + p]
        xTp = np.ascontiguousarray(
            xc.reshape(NCHUNK, NB, 32, 128).transpose(0, 3, 2, 1)
        )
        in_maps.append({"xTp": xTp, **w})

    nc = _build_bass()
    res = run_bass_kernel_spmd(nc, in_maps, core_ids=list(range(N_CORES)), trace=trace)

    y = np.empty((BATCH, FEAT), dtype=np.float32)
    for c in range(N_CORES):
        yTp = res.results[c]["yTp"]  # [n, p, r, b]
        y[c * BC:(c + 1) * BC] = (
            yTp.transpose(0, 3, 2, 1).reshape(BC, FEAT).astype(np.float32)
        )
    return y, res


def kernel(x, W_real, W_imag):
    y, _ = run_sharded(x, W_real, W_imag, trace=False)
    return y


# revision 3
# speedup vs baseline: 1.0279x; 1.0279x over previous
"""Trainium2 Bass kernel for nn_BlockFourierCirculant — v3.

Same three-matmul-stage dataflow as the baseline (batch-major spectrum with
xbar DMA transposes between stages), with the descriptor-economics fixed:

  * x / y travel HBM<->SBUF as per-chunk packed tensors with 16 KiB
    contiguous per-partition rows -> 128 descriptors per 2 MiB transfer
    (the baseline's strided layout generated 4096 512-B descriptors per
    transfer, costing ~213 us of Q7 SWDGE descriptor-generation time).
  * The xbar transposes (~10.7 us of HWDGE issue time each) are split
    across both HWDGE issuing engines (SP gets 7, ACT gets 1).
  * PSUM evacuation copies are rebalanced DVE-heavy (4:3) since ACT also
    pays transpose-issue time.

Stages per chunk of 256 batch rows:
  s1  (x stationary):  Xhat[b, slot]  = x[s, b].T @ T1[s, slot]
  xbar transpose:      (b, (g,i,u)) -> ((i,u), b)  per 4-bin group g
  s2  (X stationary):  Yhat[b, (o,u')] = Xg[(i,u), b].T @ W2[(i,u), (o,u')]
  xbar transpose:      (b, (o,kch,p4)) -> (p4, b)
  s3  (B3 stationary): y[t, b]        = B3[p4, t].T @ Y[p4, b]
"""

import numpy as np

import concourse.bacc as bacc
import concourse.bass as bass
import concourse.mybir as mybir
import concourse.tile as tile
from concourse.bass_utils import run_bass_kernel_spmd

# ---------------------------------------------------------------- constants
BATCH = 8192
FEAT = 4096
BLOCK = 256
NBLK = 16
NSLOT = 256
NGRP = 32
N_CORES = 8
BC = BATCH // N_CORES  # 1024
NB = 256
NCHUNK = BC // NB  # 4

F32 = mybir.dt.float32
F16 = mybir.dt.float16


# ------------------------------------------------------------- host matrices
def _slot_map():
    m = [(0, 0), (0, 128)]
    for k in range(1, 128):
        m.append((0, k))
        m.append((1, k))
    return m


_SLOTS = _slot_map()


def build_t1():
    s = np.arange(BLOCK)
    T1 = np.zeros((BLOCK, NSLOT), dtype=np.float64)
    for j, (comp, k) in enumerate(_SLOTS):
        ang = 2.0 * np.pi * k * s / BLOCK
        T1[:, j] = np.cos(ang) if comp == 0 else -np.sin(ang)
    return T1


def build_b3():
    t = np.arange(BLOCK)
    B3 = np.zeros((NSLOT, BLOCK), dtype=np.float64)
    for j, (comp, k) in enumerate(_SLOTS):
        w = 1.0 if k in (0, 128) else 2.0
        ang = 2.0 * np.pi * k * t / BLOCK
        B3[j] = (w * np.cos(ang) if comp == 0 else -w * np.sin(ang)) / BLOCK
    return B3


def build_w2(W_real, W_imag):
    Wr = np.asarray(W_real, dtype=np.float64)
    Wi = np.asarray(W_imag, dtype=np.float64)
    W2 = np.zeros((NGRP, 128, 128), dtype=np.float64)
    for g in range(NGRP):
        for u in range(8):
            comp_u, k_u = _SLOTS[8 * g + u]
            for up in range(8):
                comp_up, k_up = _SLOTS[8 * g + up]
                if k_u != k_up:
                    continue
                k = k_u
                if comp_u == 0 and comp_up == 0:
                    coef = Wr[:, :, k]
                elif comp_u == 1 and comp_up == 0:
                    coef = -Wi[:, :, k]
                elif comp_u == 0 and comp_up == 1:
                    coef = Wi[:, :, k]
                else:
                    coef = Wr[:, :, k]
                W2[g, u::8, up::8] = coef.T  # [i, o]
    return W2


def pack_t1(T1):
    return np.ascontiguousarray(T1.reshape(2, 128, NSLOT).transpose(1, 0, 2)).astype(
        np.float16
    )


def pack_b3(B3):
    return np.ascontiguousarray(B3.reshape(2, 128, BLOCK).transpose(1, 0, 2)).astype(
        np.float16
    )


def pack_w2(W2):
    return np.ascontiguousarray(W2.transpose(1, 0, 2)).astype(np.float16)


# ------------------------------------------------------------- device kernel
def emit_kernel(tc, outs, ins, n_chunks=NCHUNK):
    nc = tc.nc

    with (
        tc.tile_pool(name="wpool", bufs=1) as wpool,
        tc.tile_pool(name="xpool", bufs=4) as xpool,
        tc.tile_pool(name="spec", bufs=3) as spec,
        tc.tile_pool(name="yst", bufs=2) as ystp,
        tc.tile_pool(name="psum", bufs=8, space="PSUM") as pspool,
    ):
        t1_sb = wpool.tile([128, 2, NSLOT], F16)
        w2_sb = wpool.tile([128, NGRP, 128], F16)
        b3_sb = wpool.tile([128, 2, BLOCK], F16)
        # weights ride the otherwise-idle ACT HWDGE ring so they don't
        # serialize with the x(0) load on the gpsimd SWDGE queue
        nc.scalar.dma_start(out=t1_sb, in_=ins["t1"])
        nc.scalar.dma_start(out=w2_sb, in_=ins["w2"])
        nc.scalar.dma_start(out=b3_sb, in_=ins["b3"])

        ncopy = 0  # alternate evacuation copies between DVE and ACT

        def evac(dst, src):
            nonlocal ncopy
            if ncopy % 2 == 0:
                nc.vector.tensor_copy(out=dst, in_=src)
            else:
                nc.scalar.copy(out=dst, in_=src)
            ncopy += 1

        ntrans = 0  # alternate transpose issue between the two HWDGE engines

        def transpose(dst, src):
            nonlocal ntrans
            eng = nc.sync  # single ring while testing xbar concurrency safety
            eng.dma_start_transpose(out=dst, in_=src)
            ntrans += 1

        xg_t = {}
        yom_t = {}
        x_t = {}

        def load_x(n):
            # x loads are prefetched as early as the pool allows: their
            # completion sems leak (via tile's non-transitively-minimal wait
            # placement) into later transposes' wait lists, and once the xbar
            # transposes saturate the SDMA engines an in-flight x load takes
            # ~40us instead of ~7us, stalling the transpose ring spuriously.
            x_sb = xpool.tile([128, 32, NB], F16, tag="x", name=f"x_{n}")
            nc.gpsimd.dma_start(out=x_sb, in_=ins["xTp"][n])
            x_t[n] = x_sb

        def stage1(n):
            """per-block forward DFT + shuffle 1."""
            x_sb = x_t.pop(n)

            # xbt[b, bsub, g, i, u] = Xhat[b', i, slot 8g+u]
            # bsub-outer so each transpose half launches after its 8 evacs
            xbt = spec.tile([128, 2, NGRP, NBLK, 8], F16, tag="xbt", name=f"xbt_{n}")
            xg = spec.tile([128, 2, NGRP, 128], F16, tag="xg", name=f"xg_{n}")
            for bsub in range(2):
                for i in range(0, NBLK, 2):
                    ps = pspool.tile([128, 2, NSLOT], F32, tag="ps", name=f"ps1_{n}")
                    for ip in range(2):
                        for kc in range(2):
                            nc.tensor.matmul(
                                ps[:, ip, :],
                                x_sb[
                                    :, 2 * (i + ip) + kc, bsub * 128 : bsub * 128 + 128
                                ],
                                t1_sb[:, kc, :],
                                start=(kc == 0),
                                stop=(kc == 1),
                            )
                    evac(
                        xbt[:, bsub, :, i : i + 2, :],
                        ps.rearrange("p i (g u) -> p g i u", u=8),
                    )
                # shuffle 1 half: xbar transpose to spectrum-major
                transpose(xg[:, bsub], xbt[:, bsub])
            xg_t[n] = xg

        def stage2(n):
            """per-bin spectral mix + shuffle 2."""
            xg = xg_t.pop(n)
            ybt = spec.tile(
                [128, 2, NBLK, 2, 16, 8], F16, tag="xbt", name=f"ybt_{n}"
            )
            yom = spec.tile([128, 2, NBLK, 2, 128], F16, tag="xg", name=f"yom_{n}")
            for bsub in range(2):
                for g in range(0, NGRP, 4):
                    kch, gp = divmod(g, 16)
                    ps = pspool.tile([128, 4, 128], F32, tag="ps", name=f"ps2_{n}")
                    for q in range(4):
                        nc.tensor.matmul(
                            ps[:, q, :],
                            xg[:, bsub, g + q, :],
                            w2_sb[:, g + q, :],
                            start=True,
                            stop=True,
                        )
                    evac(
                        ybt[:, bsub, :, kch, gp : gp + 4, :],
                        ps.rearrange("p q (o u) -> p o q u", u=8),
                    )
                # shuffle 2 half: xbar transpose to slot-major per block
                transpose(yom[:, bsub], ybt[:, bsub])
            yom_t[n] = yom

        def stage3(n):
            """per-block inverse DFT + store."""
            yom = yom_t.pop(n)
            ybig = ystp.tile([128, 32, NB], F16, tag="ybig", name=f"ybig_{n}")
            for ob in range(0, NBLK, 4):
                pss = [
                    pspool.tile([128, 2, NB], F32, tag="ps", name=f"ps3_{n}_{ob}_{j}")
                    for j in range(4)
                ]
                for mch in range(2):
                    for kch in range(2):
                        for j in range(4):
                            nc.tensor.matmul(
                                pss[j][:, mch, :],
                                b3_sb[:, kch, mch * 128 : mch * 128 + 128],
                                yom[:, :, ob + j, kch, :],
                                start=(kch == 0),
                                stop=(kch == 1),
                            )
                for j in range(4):
                    evac(ybig[:, 2 * (ob + j) : 2 * (ob + j) + 2, :], pss[j])
            nc.gpsimd.dma_start(out=outs["yTp"][n], in_=ybig)

        # emission (= priority) order per round, hand-scheduled around the
        # transpose ring (the binding resource at ~4.9us issue per MB, which
        # the tile cost model underestimates): s1(k) first (feeds T1(k)),
        # then s2(k-1) early so T2(k-1) inputs are ready before the ring
        # reaches it, then s3(k-2) (its turn completed last round).
        # prefetch the first x chunks before any compute is emitted
        for n in range(min(4, n_chunks)):
            load_x(n)
        for k in range(n_chunks + 2):
            if k < n_chunks:
                stage1(k)
            if 0 <= k - 1 < n_chunks:
                stage2(k - 1)
            if 0 <= k - 2 < n_chunks:
                stage3(k - 2)


# ------------------------------------------------------------ host interface
_CACHED = {}


def make_inputs(W_real, W_imag):
    return {
        "t1": pack_t1(build_t1()),
        "w2": pack_w2(build_w2(W_real, W_imag)),
        "b3": pack_b3(build_b3()),
    }


def _build_bass():
    if "nc" in _CACHED:
        return _CACHED["nc"]
    nc = bacc.Bacc("TRN2", target_bir_lowering=False, debug=False)
    ins = {
        "xTp": nc.dram_tensor(
            "xTp", [NCHUNK, 128, 32, NB], F16, kind="ExternalInput"
        ).ap(),
        "t1": nc.dram_tensor("t1", [128, 2, NSLOT], F16, kind="ExternalInput").ap(),
        "w2": nc.dram_tensor("w2", [128, NGRP, 128], F16, kind="ExternalInput").ap(),
        "b3": nc.dram_tensor("b3", [128, 2, BLOCK], F16, kind="ExternalInput").ap(),
    }
    outs = {
        "yTp": nc.dram_tensor(
            "yTp", [NCHUNK, 128, 32, NB], F16, kind="ExternalOutput"
        ).ap()
    }
    with tile.TileContext(nc) as tc:
        emit_kernel(tc, outs, ins, NCHUNK)
    nc.compile()
    _CACHED["nc"] = nc
    return nc


def run_sharded(x, W_real, W_imag, trace=False):
    x = np.asarray(x, dtype=np.float32)
    w = make_inputs(W_real, W_imag)

    in_maps = []
    for c in range(N_CORES):
        xc = x[c * BC:(c + 1) * BC].astype(np.float16)  # [1024, 4096]
        # xTp[n, p, r, b] = xc[n*NB + b, r*128 + p]
        xTp = np.ascontiguousarray(
            xc.reshape(NCHUNK, NB, 32, 128).transpose(0, 3, 2, 1)
        )
        in_maps.append({"xTp": xTp, **w})

    nc = _build_bass()
    res = run_bass_kernel_spmd(nc, in_maps, core_ids=list(range(N_CORES)), trace=trace)

    y = np.empty((BATCH, FEAT), dtype=np.float32)
    for c in range(N_CORES):
        yTp = res.results[c]["yTp"]  # [n, p, r, b]
        y[c * BC:(c + 1) * BC] = (
            yTp.transpose(0, 3, 2, 1).reshape(BC, FEAT).astype(np.float32)
        )
    return y, res


def kernel(x, W_real, W_imag):
    y, _ = run_sharded(x, W_real, W_imag, trace=False)
    return y
